# revision 7
# baseline (speedup 1.0000x reference)
"""Distributed GCN (3x GCNConv + global mean pool + linear) on 8 TRN2 cores.

Sharding: nodes partitioned contiguously across 8 cores; edges partitioned by
dst owner; per-layer node features (bf16) all-gathered to a replicated table
in each core's DRAM; per-edge features fetched with dma_gather (SWDGE);
segment-sum over dst via weighted one-hot tiles GENERATED ON THE VECTOR
ENGINE per 128-edge group (tensor_scalar: (iota == slot) * weight), so no
one-hot slab is streamed from HBM.  Layers 1-2 use the reversed matmul form
(out [feat, node]) so the aggregation output directly feeds the next layer's
lhsT; layer 3 uses the forward form so pooling can consume [node, feat]
tiles via a PSUM-accumulated one-hot matmul chain (pool one-hots also
DVE-generated).  The next layer's y = h @ W matmul is fused into each
window's epilogue, with the AllGather split 80/18 windows so the big A-part
overlaps the previous layer's tail aggregation; y_full is double-buffered
across layers to avoid WAR serialization of the collective.  Gathers use
NI=1024 single-packet calls (64 descriptors per engine packet = the HW
packet ceiling) spread round-robin over the 4 SWDGE queues.
"""
import math
import numpy as np
import ml_dtypes
from contextlib import ExitStack

import concourse.bacc as bacc
import concourse.mybir as mybir
from concourse.tile import TileContext
from concourse.bass_utils import run_bass_kernel_spmd

P = 128
NCORES = 8
N = 100000
E = 1600000
H = 128
C = 10
G = 128
NP = N // NCORES            # 12500 nodes per core
NW = math.ceil(NP / P)      # 98 dst windows per core
NPAD = NW * P               # 12544 padded nodes per core
NCH = 4                     # gather chunks (int16 idx limit 32767 per chunk)
WA = 80                     # windows in the A half (AllGather split)
NPA = WA * P                # 10240 first-half nodes per core
NPB = NP - NPA              # 2260 second-half nodes per core
AREG = NCORES * NPA         # 81920 y_full rows holding all first halves
NI = 1024                   # indices per dma_gather call (64 desc/engine)
SINGLE_PACKET = True
NLAYERS = 3                 # debug knob

BF16 = ml_dtypes.bfloat16

TRACE = False               # set by test.py for profiling runs
LAST_RESULTS = {}           # debug: per-core raw results


def _wrap_idx(idx):
    """int16 gather index layout: [128, len/16], i -> [i%16, i//16], tiled x8."""
    n = idx.shape[0]
    assert n % 16 == 0
    w = idx.reshape(n // 16, 16).T.astype(np.int16)   # [16, n/16]
    return np.tile(w, (8, 1))                          # [128, n/16]


def _preprocess(edge_index):
    """Partition/sort/pad edges and build per-core gather-index and per-group
    (slot, weight) slabs for on-chip one-hot generation."""
    src0 = np.asarray(edge_index[0], dtype=np.int64)
    dst0 = np.asarray(edge_index[1], dtype=np.int64)

    deg = np.bincount(dst0, minlength=N).astype(np.float64) + 1.0
    dinv = 1.0 / np.sqrt(deg)

    # self-loop term xw * dinv^2 is handled on-device (scaled-hT matmul)
    src_a = src0
    dst_a = dst0
    w_a = (dinv[src_a] * dinv[dst_a]).astype(np.float32)

    # y_full row layout (AllGather split): [A halves of all cores | B halves]
    ids = np.arange(N, dtype=np.int64)
    coreof = ids // NP
    off = ids % NP
    newpos = np.where(off < NPA, coreof * NPA + off,
                      AREG + coreof * NPB + (off - NPA))
    gp = newpos[src_a]          # gather position of each edge's src

    owner = dst_a // NP
    wwin = (dst_a - owner * NP) // P
    slot_a = (dst_a - owner * NP - wwin * P).astype(np.int64)

    # chunk boundaries: chunks 0-2 split the A region [0, AREG) (each <=
    # 32767 rows), chunk 3 is the whole B region [AREG, N).  Grid-search the
    # two interior A split points to minimize total padded groups.
    cands = []
    for a in range(24576, 32768, 1024):
        b0 = (a + AREG) // 2
        for b in (b0 - 1024, b0, b0 + 1024):
            if b - a <= 32767 and AREG - b <= 32767 and b > a:
                cands.append([0, a, b, AREG, N])
    best = None
    for cb in cands:
        ch_c = np.searchsorted(cb[1:-1], gp, side="right")
        flat_c = ((owner * NW + wwin) * NCH + ch_c).astype(np.int64)
        cnt = np.bincount(flat_c, minlength=NCORES * NW * NCH).reshape(
            NCORES, NW, NCH)
        ng = np.ceil(cnt.max(axis=0) / P).astype(np.int64)
        tot = int(ng.sum())
        if best is None or tot < best[0]:
            best = (tot, cb, ch_c, ng)
    NG, CB, ch_a, ngrp = best
    flat = ((owner * NW + wwin) * NCH + ch_a).astype(np.int64)

    # sort: bucket-major, then src position within bucket (HBM locality)
    order_all = np.lexsort((gp, flat))
    bounds = np.searchsorted(flat[order_all], np.arange(NCORES * NW * NCH + 1))

    # per-chunk stream lengths (for gather calls): L[ch] = sum_w ngrp[w,ch]*P
    L = [int(ngrp[:, ch].sum()) * P for ch in range(NCH)]

    cores = []
    for c in range(NCORES):
        # per-chunk local row ids in (w, g) order; flat group order (w, ch, g)
        idx_parts = [[] for _ in range(NCH)]
        # flat padded stream (group-major) slot/weight for one-hot generation
        slot_stream = np.zeros(NG * P, dtype=np.int64)
        wgt_stream = np.zeros(NG * P, dtype=np.float32)
        goff = 0
        for w in range(NW):
            for ch in range(NCH):
                b = (c * NW + w) * NCH + ch
                ee = order_all[bounds[b]:bounds[b + 1]]
                k = ee.shape[0]
                npadded = int(ngrp[w, ch]) * P
                loc = np.zeros(npadded, dtype=np.int64)
                loc[:k] = gp[ee] - CB[ch]
                idx_parts[ch].append(loc)
                slot_stream[goff:goff + k] = slot_a[ee]
                wgt_stream[goff:goff + k] = w_a[ee]
                goff += npadded
        widx = np.concatenate(
            [_wrap_idx(np.concatenate(p)) if p else np.zeros((128, 0), np.int16)
             for p in idx_parts], axis=1)
        # [128, NG] slot / weight slabs: group g, edge-in-group i ->
        # slotT[i, g], wgtT[i, g]
        slotT = slot_stream.reshape(NG, P).T.astype(np.float32)
        wgtT = wgt_stream.reshape(NG, P).T.copy()
        cores.append((widx, slotT, wgtT))
    return ngrp, L, NG, CB, newpos, dinv.astype(np.float32), cores


def _build(ngrp, L, NG, CB, has_bias, has_bias2, has_blin):
    """Build the SPMD bass program (same for all cores)."""
    nc = bacc.Bacc("TRN2", num_devices=NCORES, num_swdge_queues=4,
                   dynamic_dma_scratch_size=65536)
    f32 = mybir.dt.float32
    bf16 = mybir.dt.bfloat16
    fp8 = mybir.dt.float8e4

    Loff = np.concatenate([[0], np.cumsum(L)])
    Ltot16 = int(Loff[-1]) // 16

    # ---- I/O ----
    xT = nc.dram_tensor("xT", [P, NPAD], bf16, kind="ExternalInput")
    widx_d = nc.dram_tensor("widx", [P, Ltot16], mybir.dt.int16,
                            kind="ExternalInput")
    slotT_d = nc.dram_tensor("slotT", [P, NG], f32, kind="ExternalInput")
    wgtT_d = nc.dram_tensor("wgtT", [P, NG], f32, kind="ExternalInput")
    iota_d = nc.dram_tensor("iota", [P, P], bf16, kind="ExternalInput")
    batchT_d = nc.dram_tensor("batchT", [P, NW], f32, kind="ExternalInput")
    validT_d = nc.dram_tensor("validT", [P, NW], f32, kind="ExternalInput")
    y0full_d = nc.dram_tensor("y0full", [N, H], bf16, kind="ExternalInput")
    dinv2bc_d = nc.dram_tensor("dinv2bc", [P, NPAD], bf16, kind="ExternalInput")
    cntinvb_d = nc.dram_tensor("cntinvb", [P, G], f32, kind="ExternalInput")
    Ws_d = [nc.dram_tensor(f"W{i}", [P, H], bf16, kind="ExternalInput")
            for i in range(3)]
    Wlin_d = nc.dram_tensor("Wlin", [P, C], f32, kind="ExternalInput")
    biasT_d = b2bc_d = blinb_d = None
    if has_bias:
        biasT_d = nc.dram_tensor("biasT", [P, 3], f32, kind="ExternalInput")
    if has_bias2:
        b2bc_d = nc.dram_tensor("b2bc", [P, H], f32, kind="ExternalInput")
    if has_blin:
        blinb_d = nc.dram_tensor("blinb", [P, C], f32, kind="ExternalInput")
    out_d = nc.dram_tensor("out", [G, C], f32, kind="ExternalOutput")

    y_localA = nc.dram_tensor("y_localA", [NPA, H], bf16, kind="Internal")
    y_localB = nc.dram_tensor("y_localB", [NPB, H], bf16, kind="Internal")
    # double-buffered replicated tables (one per allgathered layer) so layer
    # k+1's AllGather never WAR-serializes against layer k's gathers
    y_fulls = [nc.dram_tensor(f"y_full{i}", [N, H], bf16, kind="Internal",
                              addr_space="Shared") for i in range(2)]
    ar_in = nc.dram_tensor("ar_in", [P, G], f32, kind="Internal")
    ar_out = nc.dram_tensor("ar_out", [P, G], f32, kind="Internal",
                            addr_space="Shared")

    relu = mybir.ActivationFunctionType.Relu
    copyf = mybir.ActivationFunctionType.Copy
    iseq = mybir.AluOpType.is_equal
    mul = mybir.AluOpType.mult

    with TileContext(nc) as tc:
        with ExitStack() as ctx:
            pers = ctx.enter_context(tc.tile_pool(name="pers", bufs=1))
            sy = ctx.enter_context(tc.tile_pool(name="sy", bufs=3))
            soh = ctx.enter_context(tc.tile_pool(name="soh", bufs=6))
            sob = ctx.enter_context(tc.tile_pool(name="sob", bufs=2))
            sep = ctx.enter_context(tc.tile_pool(name="sep", bufs=3))
            gpools = [ctx.enter_context(tc.tile_pool(name=f"gat{ch}", bufs=3))
                      for ch in range(NCH)]
            psy = ctx.enter_context(tc.tile_pool(name="psy", bufs=2, space="PSUM"))
            psa = ctx.enter_context(tc.tile_pool(name="psa", bufs=3, space="PSUM"))
            psp = ctx.enter_context(tc.tile_pool(name="psp", bufs=1, space="PSUM"))

            # ---- persistent tiles ----
            hT = pers.tile([P, NPAD], bf16)
            nc.sync.dma_start(out=hT[:], in_=xT[:])
            widx = pers.tile([P, Ltot16], mybir.dt.int16)
            nc.sync.dma_start(out=widx[:], in_=widx_d[:])
            slotT = pers.tile([P, NG], f32)
            nc.sync.dma_start(out=slotT[:], in_=slotT_d[:])
            wgtT = pers.tile([P, NG], f32)
            nc.sync.dma_start(out=wgtT[:], in_=wgtT_d[:])
            iota = pers.tile([P, P], bf16)
            nc.sync.dma_start(out=iota[:], in_=iota_d[:])
            batchT = pers.tile([P, NW], f32)
            nc.sync.dma_start(out=batchT[:], in_=batchT_d[:])
            validT = pers.tile([P, NW], f32)
            nc.sync.dma_start(out=validT[:], in_=validT_d[:])
            dinv2bc = pers.tile([P, NPAD], bf16)
            nc.sync.dma_start(out=dinv2bc[:], in_=dinv2bc_d[:])
            cntinvb = pers.tile([P, G], f32)
            nc.sync.dma_start(out=cntinvb[:], in_=cntinvb_d[:])
            Ws = []
            for i in range(3):
                t = pers.tile([P, H], bf16, tag=f"W{i}")
                nc.sync.dma_start(out=t[:], in_=Ws_d[i][:])
                Ws.append(t)
            Wlin = pers.tile([P, C], f32)
            nc.sync.dma_start(out=Wlin[:], in_=Wlin_d[:])
            biasT = b2bc = blinb = None
            if has_bias:
                biasT = pers.tile([P, 3], f32)
                nc.sync.dma_start(out=biasT[:], in_=biasT_d[:])
            if has_bias2:
                b2bc = pers.tile([P, H], f32)
                nc.sync.dma_start(out=b2bc[:], in_=b2bc_d[:])
            if has_blin:
                blinb = pers.tile([P, C], f32)
                nc.sync.dma_start(out=blinb[:], in_=blinb_d[:])

            # gather call schedule per chunk: list of (start, n) within chunk
            calls = []
            for ch in range(NCH):
                cs = []
                p = 0
                while p < L[ch]:
                    n = min(NI, L[ch] - p)
                    cs.append((p, n))
                    p += n
                calls.append(cs)

            qctr = [0]          # round-robin SWDGE queue assignment

            for layer in range(NLAYERS):
                ysrc = y0full_d if layer == 0 else y_fulls[layer % 2]

                # ---- edge gather + on-chip one-hot + segment-sum matmuls ----
                cur = [-1] * NCH          # current gather call per chunk
                gtile = [None] * NCH
                pos = [0] * NCH           # consumed rows within chunk stream

                def next_group(ch, ysrc=ysrc):
                    if cur[ch] < 0 or pos[ch] >= calls[ch][cur[ch]][0] + calls[ch][cur[ch]][1]:
                        cur[ch] += 1
                        start, n = calls[ch][cur[ch]]
                        c0 = (Loff[ch] + start) // 16
                        gt = gpools[ch].tile([P, NI // P, H], bf16, tag=f"g{ch}")
                        rows0 = CB[ch]
                        rows1 = CB[ch + 1]
                        nc.gpsimd.dma_gather(
                            out_ap=gt[:, :n // P, :],
                            in_ap=ysrc[rows0:rows1],
                            idxs_ap=widx[:, c0:c0 + n // 16],
                            num_idxs=n,
                            num_idxs_reg=n,
                            elem_size=H,
                            single_packet=SINGLE_PACKET,
                            queue_num=qctr[0] % 4,
                        )
                        qctr[0] += 1
                        gtile[ch] = gt
                    start, _ = calls[ch][cur[ch]]
                    t = (pos[ch] - start) // P
                    pos[ch] += P
                    return gtile[ch][:, t, :]

                def next_oht(gcol):
                    t = soh.tile([P, P], fp8, tag="oh")
                    nc.vector.tensor_scalar(
                        out=t[:], in0=iota[:],
                        scalar1=slotT[:, gcol:gcol + 1],
                        scalar2=wgtT[:, gcol:gcol + 1],
                        op0=iseq, op1=mul)
                    return t[:]

                gcol = 0
                pool_ps = None
                if layer == NLAYERS - 1:
                    pool_ps = psp.tile([P, G], f32, space="PSUM", tag="pp")
                for w in range(NW):
                    pa = psa.tile([P, P], f32, space="PSUM", tag="pa")
                    ng_w = int(ngrp[w].sum())
                    # self-loop term: (hT_w * dinv^2) @ W opens the PSUM chain
                    hts = sep.tile([P, P], bf16, tag="hts")
                    nc.vector.tensor_tensor(
                        out=hts[:], in0=hT[:, w * P:(w + 1) * P],
                        in1=dinv2bc[:, w * P:(w + 1) * P],
                        op=mul)
                    if layer < 2:
                        # reversed: out[f, slot] = W^T(fi,f) @ hts(fi, slot)
                        nc.tensor.matmul(
                            out=pa[:], lhsT=Ws[layer][:], rhs=hts[:],
                            start=True, stop=(ng_w == 0),
                            skip_group_check=True)
                    else:
                        # forward: out[slot, f] = hts^T(fi,slot) @ W(fi, f)
                        nc.tensor.matmul(
                            out=pa[:], lhsT=hts[:], rhs=Ws[layer][:],
                            start=True, stop=(ng_w == 0),
                            skip_group_check=True)
                    done = 0
                    for ch in range(NCH):
                        for g in range(int(ngrp[w, ch])):
                            ye = next_group(ch)
                            oh = next_oht(gcol)
                            gcol += 1
                            done += 1
                            if layer < 2:
                                # reversed: out[f, slot]
                                nc.tensor.matmul(
                                    out=pa[:], lhsT=ye, rhs=oh,
                                    start=False, stop=(done == ng_w),
                                    skip_group_check=True)
                            else:
                                # forward: out[slot, f]
                                nc.tensor.matmul(
                                    out=pa[:], lhsT=oh, rhs=ye,
                                    start=False, stop=(done == ng_w),
                                    skip_group_check=True)

                    if layer < 2:
                        # epilogue on Scalar: relu(pa + b) -> hT window (bf16)
                        if has_bias:
                            nc.scalar.activation(
                                out=hT[:, w * P:(w + 1) * P], in_=pa[:],
                                func=relu, bias=biasT[:, layer:layer + 1])
                        else:
                            nc.scalar.activation(
                                out=hT[:, w * P:(w + 1) * P], in_=pa[:],
                                func=relu)
                        # fused next-layer y: y_w = h_w @ W_{layer+1}
                        py = psy.tile([P, H], f32, space="PSUM", tag="py")
                        nc.tensor.matmul(out=py[:],
                                         lhsT=hT[:, w * P:(w + 1) * P],
                                         rhs=Ws[layer + 1][:], start=True,
                                         stop=True)
                        yt = sy.tile([P, H], bf16, tag="yt")
                        nc.scalar.activation(out=yt[:], in_=py[:], func=copyf)
                        if w < WA:
                            nc.sync.dma_start(
                                out=y_localA[w * P:(w + 1) * P, :],
                                in_=yt[:])
                            if w == WA - 1:
                                nc.gpsimd.collective_compute(
                                    "AllGather", mybir.AluOpType.bypass,
                                    ins=[y_localA[:]],
                                    outs=[y_fulls[(layer + 1) % 2][0:AREG]],
                                    replica_groups=[list(range(NCORES))],
                                )
                        else:
                            r0 = (w - WA) * P
                            rows = min(NPB - r0, P)
                            nc.sync.dma_start(
                                out=y_localB[r0:r0 + rows, :],
                                in_=yt[:rows, :])
                            if w == NW - 1:
                                nc.gpsimd.collective_compute(
                                    "AllGather", mybir.AluOpType.bypass,
                                    ins=[y_localB[:]],
                                    outs=[y_fulls[(layer + 1) % 2][AREG:N]],
                                    replica_groups=[list(range(NCORES))],
                                )
                    else:
                        h3 = sep.tile([P, H], bf16, tag="h3")
                        if has_bias2:
                            hb = sep.tile([P, H], f32, tag="hb")
                            nc.vector.tensor_tensor(
                                out=hb[:], in0=pa[:], in1=b2bc[:],
                                op=mybir.AluOpType.add)
                            nc.scalar.activation(out=h3[:], in_=hb[:], func=relu)
                        else:
                            nc.scalar.activation(out=h3[:], in_=pa[:], func=relu)
                        # pooling one-hot for window w: (iota == batch) * valid
                        obt = sob.tile([P, G], bf16, tag="ob")
                        nc.vector.tensor_scalar(
                            out=obt[:], in0=iota[:],
                            scalar1=batchT[:, w:w + 1],
                            scalar2=validT[:, w:w + 1],
                            op0=iseq, op1=mul)
                        # pooling: poolT[f, g] += h3[n, f]^T @ obt[n, g]
                        nc.tensor.matmul(
                            out=pool_ps[:], lhsT=h3[:],
                            rhs=obt[:],
                            start=(w == 0), stop=(w == NW - 1),
                            skip_group_check=True)

            # ---- pooling finish ----
            poolsb = sep.tile([P, G], f32, tag="poolsb")
            nc.vector.tensor_copy(out=poolsb[:], in_=pool_ps[:])
            nc.sync.dma_start(out=ar_in[:], in_=poolsb[:])
            nc.gpsimd.collective_compute(
                "AllReduce", mybir.AluOpType.add,
                ins=[ar_in[:]], outs=[ar_out[:]],
                replica_groups=[list(range(NCORES))],
            )
            art = sep.tile([P, G], f32, tag="art")
            nc.sync.dma_start(out=art[:], in_=ar_out[:])
            ptile = sep.tile([P, G], f32, tag="ptile")
            nc.vector.tensor_tensor(out=ptile[:], in0=art[:], in1=cntinvb[:],
                                    op=mul)
            po = psy.tile([P, C], f32, space="PSUM", tag="po")
            nc.tensor.matmul(out=po[:], lhsT=ptile[:], rhs=Wlin[:],
                             start=True, stop=True)
            ot = sep.tile([P, C], f32, tag="ot")
            if has_blin:
                nc.vector.tensor_tensor(out=ot[:], in0=po[:], in1=blinb[:],
                                        op=mybir.AluOpType.add)
            else:
                nc.vector.tensor_copy(out=ot[:], in_=po[:])
            nc.sync.dma_start(out=out_d[:], in_=ot[:G, :])

    nc.compile()
    return nc


def kernel(x, edge_index, batch, W0, b0, W1, b1, W2, b2, Wlin, blin):
    x = np.asarray(x, dtype=np.float32)
    batch_np = np.asarray(batch, dtype=np.int64)
    Wl = [np.asarray(w, dtype=np.float32) for w in (W0, W1, W2)]
    bl = [np.asarray(b, dtype=np.float32) for b in (b0, b1, b2)]
    Wlin = np.asarray(Wlin, dtype=np.float32)
    blin = np.asarray(blin, dtype=np.float32)

    ngrp, L, NG, CB, newpos, dinv, cores = _preprocess(np.asarray(edge_index))
    y0 = (x @ Wl[0]).astype(BF16)      # layer-0 y precomputed host-side
    y0full = np.empty_like(y0)         # rows in y_full (A/B split) order
    y0full[newpos] = y0
    has_bias = any(np.abs(b).max() > 0 for b in bl[:2])
    has_bias2 = bool(np.abs(bl[2]).max() > 0)
    has_blin = bool(np.abs(blin).max() > 0)

    cnt = np.bincount(batch_np, minlength=G).astype(np.float32)
    cntinv = (1.0 / np.maximum(cnt, 1.0)).astype(np.float32)
    cntinvb = np.tile(cntinv[None, :], (P, 1)).astype(np.float32)  # [P, G]

    iota = np.tile(np.arange(P, dtype=np.float32)[None, :], (P, 1)).astype(BF16)

    in_maps = []
    for c in range(NCORES):
        widx, slotT, wgtT = cores[c]
        lo = c * NP
        xTa = np.zeros((P, NPAD), dtype=BF16)
        xTa[:, :NP] = x[lo:lo + NP].T.astype(BF16)
        # batch graph id / validity per (partition=node-in-window, window)
        bt = np.zeros((P, NW), dtype=np.float32)
        vt = np.zeros((P, NW), dtype=np.float32)
        pos = np.arange(NP)
        wv = pos // P
        iv = pos % P
        bt[iv, wv] = batch_np[lo:lo + NP].astype(np.float32)
        vt[iv, wv] = 1.0
        d2 = np.zeros(NPAD, dtype=np.float32)
        d2[:NP] = dinv[lo:lo + NP] ** 2
        dinv2bc = np.tile(d2[None, :], (P, 1)).astype(BF16)
        m = {
            "xT": xTa, "widx": widx, "slotT": slotT, "wgtT": wgtT,
            "iota": iota, "batchT": bt, "validT": vt,
            "y0full": y0full, "dinv2bc": dinv2bc, "cntinvb": cntinvb,
            "W0": Wl[0].astype(BF16), "W1": Wl[1].astype(BF16),
            "W2": Wl[2].astype(BF16), "Wlin": Wlin,
        }
        if has_bias:
            m["biasT"] = np.stack([bl[0], bl[1], np.zeros(H, np.float32)],
                                  axis=1).astype(np.float32)
        if has_bias2:
            m["b2bc"] = np.tile(bl[2][None, :], (P, 1)).astype(np.float32)
        if has_blin:
            m["blinb"] = np.tile(blin[None, :], (P, 1)).astype(np.float32)
        in_maps.append(m)

    nc = _build(ngrp, L, NG, CB, has_bias, has_bias2, has_blin)
    res = run_bass_kernel_spmd(nc, in_maps, core_ids=list(range(NCORES)),
                               trace=TRACE)
    global LAST_RESULTS
    LAST_RESULTS = res
    return res.results[0]["out"]


# revision 8
# speedup vs baseline: 1.6746x; 1.6746x over previous
"""Distributed GCN (3x GCNConv + global mean pool + linear) on 8 TRN2 cores.

Sharding: nodes partitioned contiguously across 8 cores; edges partitioned by
dst owner; per-layer node features (bf16) all-gathered to a replicated table
in each core's DRAM; per-edge features fetched with dma_gather (SWDGE);
segment-sum over dst via weighted one-hot tiles GENERATED ON THE VECTOR
ENGINE per 128-edge group (tensor_scalar: (iota == slot) * weight), so no
one-hot slab is streamed from HBM.  Layers 1-2 use the reversed matmul form
(out [feat, node]) so the aggregation output directly feeds the next layer's
lhsT; layer 3 uses the forward form so pooling can consume [node, feat]
tiles via a PSUM-accumulated one-hot matmul chain (pool one-hots also
DVE-generated).  The next layer's y = h @ W matmul is fused into each
window's epilogue, with the AllGather split 80/18 windows so the big A-part
overlaps the previous layer's tail aggregation; y_full is double-buffered
across layers to avoid WAR serialization of the collective.  Gathers use
NI=1024 single-packet calls (64 descriptors per engine packet = the HW
packet ceiling) spread round-robin over the 4 SWDGE queues.
"""
import math
import numpy as np
import ml_dtypes
from contextlib import ExitStack

import concourse.bacc as bacc
import concourse.mybir as mybir
from concourse.tile import TileContext
from concourse.bass_utils import run_bass_kernel_spmd

P = 128
NCORES = 8
N = 100000
E = 1600000
H = 128
C = 10
G = 128
NP = N // NCORES            # 12500 nodes per core
NW = math.ceil(NP / P)      # 98 dst windows per core
NPAD = NW * P               # 12544 padded nodes per core
NCH = 4                     # gather chunks (int16 idx limit 32767 per chunk)
WA = 80                     # windows in the A half (AllGather split)
NPA = WA * P                # 10240 first-half nodes per core
NPB = NP - NPA              # 2260 second-half nodes per core
AREG = NCORES * NPA         # 81920 y_full rows holding all first halves
NI = 1024                   # indices per dma_gather call (64 desc/engine)
SINGLE_PACKET = True
NLAYERS = 3                 # debug knob

BF16 = ml_dtypes.bfloat16
FP8 = ml_dtypes.float8_e4m3
OHK = 16                    # one-hot groups fetched per DMA

TRACE = False               # set by test.py for profiling runs
LAST_RESULTS = {}           # debug: per-core raw results


def _wrap_idx(idx):
    """int16 gather index layout: [128, len/16], i -> [i%16, i//16], tiled x8."""
    n = idx.shape[0]
    assert n % 16 == 0
    w = idx.reshape(n // 16, 16).T.astype(np.int16)   # [16, n/16]
    return np.tile(w, (8, 1))                          # [128, n/16]


def _preprocess(edge_index):
    """Partition/sort/pad edges and build per-core gather-index and per-group
    (slot, weight) slabs for on-chip one-hot generation."""
    src0 = np.asarray(edge_index[0], dtype=np.int64)
    dst0 = np.asarray(edge_index[1], dtype=np.int64)

    deg = np.bincount(dst0, minlength=N).astype(np.float64) + 1.0
    dinv = 1.0 / np.sqrt(deg)

    # self-loop term xw * dinv^2 is handled on-device (scaled-hT matmul)
    src_a = src0
    dst_a = dst0
    w_a = (dinv[src_a] * dinv[dst_a]).astype(np.float32)

    # y_full row layout (AllGather split): [A halves of all cores | B halves]
    ids = np.arange(N, dtype=np.int64)
    coreof = ids // NP
    off = ids % NP
    newpos = np.where(off < NPA, coreof * NPA + off,
                      AREG + coreof * NPB + (off - NPA))
    gp = newpos[src_a]          # gather position of each edge's src

    owner = dst_a // NP
    wwin = (dst_a - owner * NP) // P
    slot_a = (dst_a - owner * NP - wwin * P).astype(np.int64)

    # chunk boundaries: chunks 0-2 split the A region [0, AREG) (each <=
    # 32767 rows), chunk 3 is the whole B region [AREG, N).  Grid-search the
    # two interior A split points to minimize total padded groups.
    cands = []
    for a in range(24576, 32768, 1024):
        b0 = (a + AREG) // 2
        for b in (b0 - 1024, b0, b0 + 1024):
            if b - a <= 32767 and AREG - b <= 32767 and b > a:
                cands.append([0, a, b, AREG, N])
    best = None
    for cb in cands:
        ch_c = np.searchsorted(cb[1:-1], gp, side="right")
        flat_c = ((owner * NW + wwin) * NCH + ch_c).astype(np.int64)
        cnt = np.bincount(flat_c, minlength=NCORES * NW * NCH).reshape(
            NCORES, NW, NCH)
        ng = np.ceil(cnt.max(axis=0) / P).astype(np.int64)
        tot = int(ng.sum())
        if best is None or tot < best[0]:
            best = (tot, cb, ch_c, ng)
    NG, CB, ch_a, ngrp = best
    flat = ((owner * NW + wwin) * NCH + ch_a).astype(np.int64)

    # sort: bucket-major, then src position within bucket (HBM locality)
    order_all = np.lexsort((gp, flat))
    bounds = np.searchsorted(flat[order_all], np.arange(NCORES * NW * NCH + 1))

    # per-chunk stream lengths (for gather calls): L[ch] = sum_w ngrp[w,ch]*P
    L = [int(ngrp[:, ch].sum()) * P for ch in range(NCH)]

    cores = []
    for c in range(NCORES):
        # per-chunk local row ids in (w, g) order; flat group order (w, ch, g)
        idx_parts = [[] for _ in range(NCH)]
        # flat padded stream (group-major) slot/weight for one-hot generation
        slot_stream = np.zeros(NG * P, dtype=np.int64)
        wgt_stream = np.zeros(NG * P, dtype=np.float32)
        goff = 0
        for w in range(NW):
            for ch in range(NCH):
                b = (c * NW + w) * NCH + ch
                ee = order_all[bounds[b]:bounds[b + 1]]
                k = ee.shape[0]
                npadded = int(ngrp[w, ch]) * P
                loc = np.zeros(npadded, dtype=np.int64)
                loc[:k] = gp[ee] - CB[ch]
                idx_parts[ch].append(loc)
                slot_stream[goff:goff + k] = slot_a[ee]
                wgt_stream[goff:goff + k] = w_a[ee]
                goff += npadded
        widx = np.concatenate(
            [_wrap_idx(np.concatenate(p)) if p else np.zeros((128, 0), np.int16)
             for p in idx_parts], axis=1)
        # weighted one-hot slab [P, NG*P]: stream position pos = g*P + i ->
        # partition i, column g*P + slot
        pos = np.arange(NG * P)
        part = pos % P
        grp = pos // P
        ohw = np.zeros((P, NG * P), dtype=FP8)
        ohw[part, grp * P + slot_stream] = wgt_stream.astype(FP8)
        cores.append((widx, ohw))
    return ngrp, L, NG, CB, newpos, dinv.astype(np.float32), cores


def _build(ngrp, L, NG, CB, has_bias, has_bias2, has_blin):
    """Build the SPMD bass program (same for all cores)."""
    nc = bacc.Bacc("TRN2", num_devices=NCORES, num_swdge_queues=4,
                   dynamic_dma_scratch_size=65536)
    f32 = mybir.dt.float32
    bf16 = mybir.dt.bfloat16
    fp8 = mybir.dt.float8e4

    Loff = np.concatenate([[0], np.cumsum(L)])
    Ltot16 = int(Loff[-1]) // 16

    # ---- I/O ----
    xT = nc.dram_tensor("xT", [P, NPAD], bf16, kind="ExternalInput")
    widx_d = nc.dram_tensor("widx", [P, Ltot16], mybir.dt.int16,
                            kind="ExternalInput")
    ohw_d = nc.dram_tensor("ohw", [P, NG * P], fp8, kind="ExternalInput")
    iota_d = nc.dram_tensor("iota", [P, P], bf16, kind="ExternalInput")
    batchT_d = nc.dram_tensor("batchT", [P, NW], f32, kind="ExternalInput")
    validT_d = nc.dram_tensor("validT", [P, NW], f32, kind="ExternalInput")
    y0full_d = nc.dram_tensor("y0full", [N, H], bf16, kind="ExternalInput")
    dinv2bc_d = nc.dram_tensor("dinv2bc", [P, NPAD], bf16, kind="ExternalInput")
    cntinvb_d = nc.dram_tensor("cntinvb", [P, G], f32, kind="ExternalInput")
    Ws_d = [nc.dram_tensor(f"W{i}", [P, H], bf16, kind="ExternalInput")
            for i in range(3)]
    Wlin_d = nc.dram_tensor("Wlin", [P, C], f32, kind="ExternalInput")
    biasT_d = b2bc_d = blinb_d = None
    if has_bias:
        biasT_d = nc.dram_tensor("biasT", [P, 3], f32, kind="ExternalInput")
    if has_bias2:
        b2bc_d = nc.dram_tensor("b2bc", [P, H], f32, kind="ExternalInput")
    if has_blin:
        blinb_d = nc.dram_tensor("blinb", [P, C], f32, kind="ExternalInput")
    out_d = nc.dram_tensor("out", [G, C], f32, kind="ExternalOutput")

    y_localA = nc.dram_tensor("y_localA", [NPA, H], bf16, kind="Internal")
    y_localB = nc.dram_tensor("y_localB", [NPB, H], bf16, kind="Internal")
    # double-buffered replicated tables (one per allgathered layer) so layer
    # k+1's AllGather never WAR-serializes against layer k's gathers
    y_fulls = [nc.dram_tensor(f"y_full{i}", [N, H], bf16, kind="Internal",
                              addr_space="Shared") for i in range(2)]
    ar_in = nc.dram_tensor("ar_in", [P, G], f32, kind="Internal")
    ar_out = nc.dram_tensor("ar_out", [P, G], f32, kind="Internal",
                            addr_space="Shared")

    relu = mybir.ActivationFunctionType.Relu
    copyf = mybir.ActivationFunctionType.Copy
    iseq = mybir.AluOpType.is_equal
    mul = mybir.AluOpType.mult

    with TileContext(nc) as tc:
        with ExitStack() as ctx:
            pers = ctx.enter_context(tc.tile_pool(name="pers", bufs=1))
            sy = ctx.enter_context(tc.tile_pool(name="sy", bufs=3))
            soh = ctx.enter_context(tc.tile_pool(name="soh", bufs=3))
            sob = ctx.enter_context(tc.tile_pool(name="sob", bufs=2))
            sep = ctx.enter_context(tc.tile_pool(name="sep", bufs=3))
            gpools = [ctx.enter_context(tc.tile_pool(name=f"gat{ch}", bufs=3))
                      for ch in range(NCH)]
            psy = ctx.enter_context(tc.tile_pool(name="psy", bufs=2, space="PSUM"))
            psa = ctx.enter_context(tc.tile_pool(name="psa", bufs=3, space="PSUM"))
            psp = ctx.enter_context(tc.tile_pool(name="psp", bufs=1, space="PSUM"))

            # ---- persistent tiles ----
            hT = pers.tile([P, NPAD], bf16)
            nc.sync.dma_start(out=hT[:], in_=xT[:])
            widx = pers.tile([P, Ltot16], mybir.dt.int16)
            nc.sync.dma_start(out=widx[:], in_=widx_d[:])
            iota = pers.tile([P, P], bf16)
            nc.sync.dma_start(out=iota[:], in_=iota_d[:])
            batchT = pers.tile([P, NW], f32)
            nc.sync.dma_start(out=batchT[:], in_=batchT_d[:])
            validT = pers.tile([P, NW], f32)
            nc.sync.dma_start(out=validT[:], in_=validT_d[:])
            dinv2bc = pers.tile([P, NPAD], bf16)
            nc.sync.dma_start(out=dinv2bc[:], in_=dinv2bc_d[:])
            cntinvb = pers.tile([P, G], f32)
            nc.sync.dma_start(out=cntinvb[:], in_=cntinvb_d[:])
            Ws = []
            for i in range(3):
                t = pers.tile([P, H], bf16, tag=f"W{i}")
                nc.sync.dma_start(out=t[:], in_=Ws_d[i][:])
                Ws.append(t)
            Wlin = pers.tile([P, C], f32)
            nc.sync.dma_start(out=Wlin[:], in_=Wlin_d[:])
            biasT = b2bc = blinb = None
            if has_bias:
                biasT = pers.tile([P, 3], f32)
                nc.sync.dma_start(out=biasT[:], in_=biasT_d[:])
            if has_bias2:
                b2bc = pers.tile([P, H], f32)
                nc.sync.dma_start(out=b2bc[:], in_=b2bc_d[:])
            if has_blin:
                blinb = pers.tile([P, C], f32)
                nc.sync.dma_start(out=blinb[:], in_=blinb_d[:])

            # gather call schedule per chunk: list of (start, n) within chunk
            calls = []
            for ch in range(NCH):
                cs = []
                p = 0
                while p < L[ch]:
                    n = min(NI, L[ch] - p)
                    cs.append((p, n))
                    p += n
                calls.append(cs)

            qctr = [0]          # round-robin SWDGE queue assignment

            for layer in range(NLAYERS):
                ysrc = y0full_d if layer == 0 else y_fulls[layer % 2]

                # ---- edge gather + on-chip one-hot + segment-sum matmuls ----
                cur = [-1] * NCH          # current gather call per chunk
                gtile = [None] * NCH
                pos = [0] * NCH           # consumed rows within chunk stream

                def next_group(ch, ysrc=ysrc):
                    if cur[ch] < 0 or pos[ch] >= calls[ch][cur[ch]][0] + calls[ch][cur[ch]][1]:
                        cur[ch] += 1
                        start, n = calls[ch][cur[ch]]
                        c0 = (Loff[ch] + start) // 16
                        gt = gpools[ch].tile([P, NI // P, H], bf16, tag=f"g{ch}")
                        rows0 = CB[ch]
                        rows1 = CB[ch + 1]
                        nc.gpsimd.dma_gather(
                            out_ap=gt[:, :n // P, :],
                            in_ap=ysrc[rows0:rows1],
                            idxs_ap=widx[:, c0:c0 + n // 16],
                            num_idxs=n,
                            num_idxs_reg=n,
                            elem_size=H,
                            single_packet=SINGLE_PACKET,
                            queue_num=qctr[0] % 4,
                        )
                        qctr[0] += 1
                        gtile[ch] = gt
                    start, _ = calls[ch][cur[ch]]
                    t = (pos[ch] - start) // P
                    pos[ch] += P
                    return gtile[ch][:, t, :]

                ohw_cur = [-1]
                ohw_tile = [None]

                def next_ohw(gcol):
                    blk = gcol // OHK
                    if blk != ohw_cur[0]:
                        ohw_cur[0] = blk
                        c0 = blk * OHK * P
                        w_cols = min(OHK * P, NG * P - c0)
                        t = soh.tile([P, OHK * P], fp8, tag="oh")
                        nc.sync.dma_start(out=t[:, :w_cols],
                                          in_=ohw_d[:, c0:c0 + w_cols])
                        ohw_tile[0] = t
                    o = gcol % OHK
                    return ohw_tile[0][:, o * P:(o + 1) * P]

                gcol = 0
                pool_ps = None
                if layer == NLAYERS - 1:
                    pool_ps = psp.tile([P, G], f32, space="PSUM", tag="pp")
                for w in range(NW):
                    pa = psa.tile([P, P], f32, space="PSUM", tag="pa")
                    ng_w = int(ngrp[w].sum())
                    # self-loop term: (hT_w * dinv^2) @ W opens the PSUM chain
                    hts = sep.tile([P, P], bf16, tag="hts")
                    nc.vector.tensor_tensor(
                        out=hts[:], in0=hT[:, w * P:(w + 1) * P],
                        in1=dinv2bc[:, w * P:(w + 1) * P],
                        op=mul)
                    if layer < 2:
                        # reversed: out[f, slot] = W^T(fi,f) @ hts(fi, slot)
                        nc.tensor.matmul(
                            out=pa[:], lhsT=Ws[layer][:], rhs=hts[:],
                            start=True, stop=(ng_w == 0),
                            skip_group_check=True)
                    else:
                        # forward: out[slot, f] = hts^T(fi,slot) @ W(fi, f)
                        nc.tensor.matmul(
                            out=pa[:], lhsT=hts[:], rhs=Ws[layer][:],
                            start=True, stop=(ng_w == 0),
                            skip_group_check=True)
                    done = 0
                    for ch in range(NCH):
                        for g in range(int(ngrp[w, ch])):
                            ye = next_group(ch)
                            oh = next_ohw(gcol)
                            gcol += 1
                            done += 1
                            if layer < 2:
                                # reversed: out[f, slot]
                                nc.tensor.matmul(
                                    out=pa[:], lhsT=ye, rhs=oh,
                                    start=False, stop=(done == ng_w),
                                    skip_group_check=True)
                            else:
                                # forward: out[slot, f]
                                nc.tensor.matmul(
                                    out=pa[:], lhsT=oh, rhs=ye,
                                    start=False, stop=(done == ng_w),
                                    skip_group_check=True)

                    if layer < 2:
                        # epilogue on Scalar: relu(pa + b) -> hT window (bf16)
                        if has_bias:
                            nc.scalar.activation(
                                out=hT[:, w * P:(w + 1) * P], in_=pa[:],
                                func=relu, bias=biasT[:, layer:layer + 1])
                        else:
                            nc.scalar.activation(
                                out=hT[:, w * P:(w + 1) * P], in_=pa[:],
                                func=relu)
                        # fused next-layer y: y_w = h_w @ W_{layer+1}
                        py = psy.tile([P, H], f32, space="PSUM", tag="py")
                        nc.tensor.matmul(out=py[:],
                                         lhsT=hT[:, w * P:(w + 1) * P],
                                         rhs=Ws[layer + 1][:], start=True,
                                         stop=True)
                        yt = sy.tile([P, H], bf16, tag="yt")
                        nc.scalar.activation(out=yt[:], in_=py[:], func=copyf)
                        if w < WA:
                            nc.sync.dma_start(
                                out=y_localA[w * P:(w + 1) * P, :],
                                in_=yt[:])
                            if w == WA - 1:
                                nc.gpsimd.collective_compute(
                                    "AllGather", mybir.AluOpType.bypass,
                                    ins=[y_localA[:]],
                                    outs=[y_fulls[(layer + 1) % 2][0:AREG]],
                                    replica_groups=[list(range(NCORES))],
                                )
                        else:
                            r0 = (w - WA) * P
                            rows = min(NPB - r0, P)
                            nc.sync.dma_start(
                                out=y_localB[r0:r0 + rows, :],
                                in_=yt[:rows, :])
                            if w == NW - 1:
                                nc.gpsimd.collective_compute(
                                    "AllGather", mybir.AluOpType.bypass,
                                    ins=[y_localB[:]],
                                    outs=[y_fulls[(layer + 1) % 2][AREG:N]],
                                    replica_groups=[list(range(NCORES))],
                                )
                    else:
                        h3 = sep.tile([P, H], bf16, tag="h3")
                        if has_bias2:
                            hb = sep.tile([P, H], f32, tag="hb")
                            nc.vector.tensor_tensor(
                                out=hb[:], in0=pa[:], in1=b2bc[:],
                                op=mybir.AluOpType.add)
                            nc.scalar.activation(out=h3[:], in_=hb[:], func=relu)
                        else:
                            nc.scalar.activation(out=h3[:], in_=pa[:], func=relu)
                        # pooling one-hot for window w: (iota == batch) * valid
                        obt = sob.tile([P, G], bf16, tag="ob")
                        nc.vector.tensor_scalar(
                            out=obt[:], in0=iota[:],
                            scalar1=batchT[:, w:w + 1],
                            scalar2=validT[:, w:w + 1],
                            op0=iseq, op1=mul)
                        # pooling: poolT[f, g] += h3[n, f]^T @ obt[n, g]
                        nc.tensor.matmul(
                            out=pool_ps[:], lhsT=h3[:],
                            rhs=obt[:],
                            start=(w == 0), stop=(w == NW - 1),
                            skip_group_check=True)

            # ---- pooling finish ----
            poolsb = sep.tile([P, G], f32, tag="poolsb")
            nc.vector.tensor_copy(out=poolsb[:], in_=pool_ps[:])
            nc.sync.dma_start(out=ar_in[:], in_=poolsb[:])
            nc.gpsimd.collective_compute(
                "AllReduce", mybir.AluOpType.add,
                ins=[ar_in[:]], outs=[ar_out[:]],
                replica_groups=[list(range(NCORES))],
            )
            art = sep.tile([P, G], f32, tag="art")
            nc.sync.dma_start(out=art[:], in_=ar_out[:])
            ptile = sep.tile([P, G], f32, tag="ptile")
            nc.vector.tensor_tensor(out=ptile[:], in0=art[:], in1=cntinvb[:],
                                    op=mul)
            po = psy.tile([P, C], f32, space="PSUM", tag="po")
            nc.tensor.matmul(out=po[:], lhsT=ptile[:], rhs=Wlin[:],
                             start=True, stop=True)
            ot = sep.tile([P, C], f32, tag="ot")
            if has_blin:
                nc.vector.tensor_tensor(out=ot[:], in0=po[:], in1=blinb[:],
                                        op=mybir.AluOpType.add)
            else:
                nc.vector.tensor_copy(out=ot[:], in_=po[:])
            nc.sync.dma_start(out=out_d[:], in_=ot[:G, :])

    nc.compile()
    return nc


def kernel(x, edge_index, batch, W0, b0, W1, b1, W2, b2, Wlin, blin):
    x = np.asarray(x, dtype=np.float32)
    batch_np = np.asarray(batch, dtype=np.int64)
    Wl = [np.asarray(w, dtype=np.float32) for w in (W0, W1, W2)]
    bl = [np.asarray(b, dtype=np.float32) for b in (b0, b1, b2)]
    Wlin = np.asarray(Wlin, dtype=np.float32)
    blin = np.asarray(blin, dtype=np.float32)

    ngrp, L, NG, CB, newpos, dinv, cores = _preprocess(np.asarray(edge_index))
    y0 = (x @ Wl[0]).astype(BF16)      # layer-0 y precomputed host-side
    y0full = np.empty_like(y0)         # rows in y_full (A/B split) order
    y0full[newpos] = y0
    has_bias = any(np.abs(b).max() > 0 for b in bl[:2])
    has_bias2 = bool(np.abs(bl[2]).max() > 0)
    has_blin = bool(np.abs(blin).max() > 0)

    cnt = np.bincount(batch_np, minlength=G).astype(np.float32)
    cntinv = (1.0 / np.maximum(cnt, 1.0)).astype(np.float32)
    cntinvb = np.tile(cntinv[None, :], (P, 1)).astype(np.float32)  # [P, G]

    iota = np.tile(np.arange(P, dtype=np.float32)[None, :], (P, 1)).astype(BF16)

    in_maps = []
    for c in range(NCORES):
        widx, ohw = cores[c]
        lo = c * NP
        xTa = np.zeros((P, NPAD), dtype=BF16)
        xTa[:, :NP] = x[lo:lo + NP].T.astype(BF16)
        # batch graph id / validity per (partition=node-in-window, window)
        bt = np.zeros((P, NW), dtype=np.float32)
        vt = np.zeros((P, NW), dtype=np.float32)
        pos = np.arange(NP)
        wv = pos // P
        iv = pos % P
        bt[iv, wv] = batch_np[lo:lo + NP].astype(np.float32)
        vt[iv, wv] = 1.0
        d2 = np.zeros(NPAD, dtype=np.float32)
        d2[:NP] = dinv[lo:lo + NP] ** 2
        dinv2bc = np.tile(d2[None, :], (P, 1)).astype(BF16)
        m = {
            "xT": xTa, "widx": widx, "ohw": ohw,
            "iota": iota, "batchT": bt, "validT": vt,
            "y0full": y0full, "dinv2bc": dinv2bc, "cntinvb": cntinvb,
            "W0": Wl[0].astype(BF16), "W1": Wl[1].astype(BF16),
            "W2": Wl[2].astype(BF16), "Wlin": Wlin,
        }
        if has_bias:
            m["biasT"] = np.stack([bl[0], bl[1], np.zeros(H, np.float32)],
                                  axis=1).astype(np.float32)
        if has_bias2:
            m["b2bc"] = np.tile(bl[2][None, :], (P, 1)).astype(np.float32)
        if has_blin:
            m["blinb"] = np.tile(blin[None, :], (P, 1)).astype(np.float32)
        in_maps.append(m)

    nc = _build(ngrp, L, NG, CB, has_bias, has_bias2, has_blin)
    res = run_bass_kernel_spmd(nc, in_maps, core_ids=list(range(NCORES)),
                               trace=TRACE)
    global LAST_RESULTS
    LAST_RESULTS = res
    return res.results[0]["out"]
